# revision 18
# baseline (speedup 1.0000x reference)
"""Chunked causal self-attention with RoPE on 8 Trainium2 NeuronCores.

Problem: B=4, L=4096, H=16, DH=DV=128, CHUNK=1024 (N=4 chunks).
RoPE on q,k then chunk-local causal attention per (batch, chunk, head).

Sharding: heads split across 8 cores (2 heads/core) -> 32 independent
(1024 x 1024, d=128) attention problems per core, grouped 4-per-(b,h)
so one load covers a whole group.

v3 design notes (from trace analysis of v2: DVE was 89.5% busy on the
softmax-denominator R-build + fp32 PSUM drains; ACT 69%, PE 72%):
  - Fused denominator: PV runs transposed, out[q,dv] = sum_k P[k,q] *
    Vext[k,dv] with Vext = [V | ones]; the 129th output column IS the
    softmax denominator. This deletes the DVE R-build, the PE den
    matmuls and the separate den drain.
  - exp in 4 activations/problem (block units {b0,b4} {b1,b7} {b2,b3}
    {b5,b6} = 1536/1024/1408/640 cols) to amortize ScalarE overhead.
  - PSUM: psA [128,1536] (3 banks) + psB [128,1024] (2) ping-pong for
    scores; psO [128,1536] (3) holds the 8 q-block PV outputs at
    offsets {0,129,258, 512,641,770, 1024,1153} so no matmul output
    crosses a bank. Total exactly 8 banks.
  - DVE only does 8 diag masks + 3 compacting PSUM->SBUF casts per
    problem (~3us vs ~7us in v2).
  - Normalization (num/den) and final layout transposes on host.
"""

import math
import os
import sys

import numpy as np

for _p in ("/opt/trn_rl_repo", "/root/.axon_site/_ro/trn_rl_repo"):
    if os.path.isdir(_p) and _p not in sys.path:
        sys.path.insert(0, _p)

import concourse.bass as bass  # noqa: E402
import concourse.tile as tile  # noqa: E402
from concourse import bass_utils, mybir  # noqa: E402

B, L, H, DH, DV = 4, 4096, 16, 128, 128
CHUNK = 1024
NCHUNK = L // CHUNK  # 4
ROPE_BASE = 10000.0
NCORES = 8
HPC = H // NCORES  # 2 heads per core
NPROB = B * HPC * NCHUNK  # 32 problems per core
NG = B * HPC  # 8 groups of 4 chunks
HALF = DH // 2  # 64
NB = CHUNK // 128  # 8 k-blocks
SCALE = 1.0 / math.sqrt(DH)
VW = 130  # v block width fed to PV: 128 dv + ones col (den) + zero pad
VS = 132  # v block stride in SBUF/HBM (8B-aligned: 132*2B = 264B)
PW = NB * VS  # 1056 packed v cols per problem
OW = 130  # psO q-block region stride (130*4B = 520B, 8B-aligned)
POW = NB * OW  # 1040 packed output cols per problem

F16 = mybir.dt.float16
F32 = mybir.dt.float32
AF = mybir.ActivationFunctionType

# exp units: (tag, psum pool key, [(block, tile_col_off)], span)
# unit tile sizes: A=1536 (3 banks), B=1024 (2 banks); A and B ping-pong.
UNITS = [
    ("uA", "A", [(0, 0), (4, 1024)], 1536),
    ("uB", "B", [(1, 0), (7, 896)], 1024),
    ("uC", "A", [(3, 0), (5, 640)], 1024),
    ("uD", "B", [(2, 0), (6, 768)], 1024),
]
# block -> (unit index, tile col offset of block start)
BLK = {}
for _ui, (_, _, _blocks, _) in enumerate(UNITS):
    for _b, _off in _blocks:
        BLK[_b] = (_ui, _off)

# psO column offset of each q-block's 129-wide output region (bank safe:
# banks hold 512 fp32; 3+3+2 regions per bank; 8B-aligned starts).
QB_OFF = [0, 130, 260, 512, 642, 772, 1024, 1154]
# drain: (psO src range) -> (outg dst offset); keeps the 130-col stride
DRAINS = [(0, 390, 0), (512, 902, 390), (1024, 1284, 780)]


def _block_region(b):
    """absolute q range covered for k-block b (causal)."""
    return 128 * b, CHUNK


def build_module(nprob=NPROB):
    from concourse import bacc

    nc = bacc.Bacc("TRN2", target_bir_lowering=False, debug=False)

    qT = nc.dram_tensor("qT_in", (128, nprob * CHUNK), F16, kind="ExternalInput")
    kT = nc.dram_tensor("kT_in", (128, nprob * CHUNK), F16, kind="ExternalInput")
    vT = nc.dram_tensor("vT_in", (128, nprob * PW), F16, kind="ExternalInput")
    tri = nc.dram_tensor("tri_in", (128, 128), F16, kind="ExternalInput")

    outT = nc.dram_tensor("outT_out", (128, nprob * POW), F16, kind="ExternalOutput")

    with tile.TileContext(nc) as tc:
        _body(tc, nprob, qT, kT, vT, tri, outT)
    nc.compile()
    return nc


def _body(tc, nprob, qT, kT, vT, tri, outT):
    from contextlib import ExitStack

    nc = tc.nc
    ngroups = nprob // NCHUNK
    GW = NCHUNK * CHUNK  # q/k group width: 4096 cols
    GV = NCHUNK * PW  # v group width: 4224 cols
    GO = NCHUNK * POW  # out group width: 4160 cols

    with ExitStack() as ctx:
        consts = ctx.enter_context(tc.tile_pool(name="consts", bufs=1))
        ing = ctx.enter_context(tc.tile_pool(name="ing", bufs=2))
        ptp = ctx.enter_context(tc.tile_pool(name="ptp", bufs=2))
        outp = ctx.enter_context(tc.tile_pool(name="outp", bufs=2))
        psAp = ctx.enter_context(tc.tile_pool(name="psA", bufs=1, space="PSUM"))
        psBp = ctx.enter_context(tc.tile_pool(name="psB", bufs=1, space="PSUM"))
        psOp = ctx.enter_context(tc.tile_pool(name="psO", bufs=1, space="PSUM"))

        tri_t = consts.tile([128, 128], F16, tag="tri")
        nc.sync.dma_start(out=tri_t, in_=tri.ap())

        # touch consts once so compute ops don't carry extra DMA waits
        dummy = consts.tile([128, 1], F16, tag="dummy")
        nc.vector.tensor_copy(out=dummy, in_=tri_t[:, 0:1])

        state = {}

        def emit_loads(g, fine=False):
            # split DMAs so early problems' slices land first (subtile
            # deps let their scores start before the rest arrives)
            qg = ing.tile([128, GW], F16, tag="qg")
            kg = ing.tile([128, GW], F16, tag="kg")
            vg = ing.tile([128, GV], F16, tag="vg")
            pieces = range(NCHUNK) if fine else [0, 1]
            for i in pieces:
                # q/k piece i covers chunk i (fine) or chunk 0 / rest
                if fine:
                    qs, qe = i * CHUNK, (i + 1) * CHUNK
                    vs_, ve = i * PW, (i + 1) * PW
                else:
                    qs, qe = (0, CHUNK) if i == 0 else (CHUNK, GW)
                    vs_, ve = (0, PW) if i == 0 else (PW, GV)
                nc.sync.dma_start(out=qg[:, qs:qe],
                                  in_=qT.ap()[:, g * GW + qs:g * GW + qe])
                nc.sync.dma_start(out=kg[:, qs:qe],
                                  in_=kT.ap()[:, g * GW + qs:g * GW + qe])
                nc.sync.dma_start(out=vg[:, vs_:ve],
                                  in_=vT.ap()[:, g * GV + vs_:g * GV + ve])
            state[("grp", g)] = (qg, kg, vg)

        def emit_scores_exp_unit(p, ui):
            """score matmuls + one exp for unit ui of problem p."""
            g, pi = divmod(p, NCHUNK)
            qg, kg, vg = state[("grp", g)]
            poff = pi * CHUNK
            tag, pool_key, blocks, span = UNITS[ui]
            pool = psAp if pool_key == "A" else psBp
            ps = pool.tile([128, 1536 if pool_key == "A" else 1024], F32,
                           name=f"ps{pool_key}", tag=f"ps{pool_key}")
            for b, off in blocks:
                q0, q1 = _block_region(b)
                kblk = kg[:, poff + 128 * b: poff + 128 * (b + 1)]
                # split matmuls at psum bank boundaries (512 fp32 cols),
                # in tile coords: block cols live at [off, off + q1-q0)
                a = q0
                while a < q1:
                    ta = a - q0 + off
                    bank_end = ((ta // 512) + 1) * 512
                    e = min(q1, a + (bank_end - ta))
                    nc.tensor.matmul(
                        ps[:, ta:ta + (e - a)],
                        lhsT=kblk,
                        rhs=qg[:, poff + a: poff + e],
                        start=True, stop=True,
                    )
                    a = e
            pt = ptp.tile([128, span], F16, name=f"pt{tag}", tag=f"pt{tag}")
            nc.scalar.activation(
                out=pt, in_=ps[:, 0:span], func=AF.Exp, scale=SCALE,
            )
            state[("pt", p, ui)] = pt

        def emit_masks(p, uis):
            """diagonal masks for the blocks of units uis (DVE, in-place)."""
            for ui in uis:
                _, _, blocks, _ = UNITS[ui]
                pt = state[("pt", p, ui)]
                for b, off in blocks:
                    nc.vector.tensor_mul(
                        pt[:, off:off + 128], pt[:, off:off + 128], tri_t
                    )

        def emit_pv(p, qbs):
            """transposed PV with fused denominator: for each q-block qb,
            out[q, 0:130] = sum_{kb<=qb} P_kb[:, qb].T @ [V_kb | 1 | 0]."""
            g, pi = divmod(p, NCHUNK)
            qg, kg, vg = state[("grp", g)]
            voff = pi * PW
            if qbs[0] == 0:
                state[("psO", p)] = psOp.tile(
                    [128, 1536], F32, name="pso", tag="psO"
                )
            pso = state[("psO", p)]
            for qb in qbs:
                o0 = QB_OFF[qb]
                for kb in range(qb + 1):
                    ui, off = BLK[kb]
                    pt = state[("pt", p, ui)]
                    # lhsT: P columns for q-block qb within block kb
                    c0 = off + 128 * (qb - kb)
                    nc.tensor.matmul(
                        pso[:, o0:o0 + VW],
                        lhsT=pt[:, c0:c0 + 128],
                        rhs=vg[:, voff + VS * kb: voff + VS * kb + VW],
                        start=(kb == 0), stop=(kb == qb),
                    )
            if qbs[-1] == NB - 1:
                for ui in range(4):
                    state.pop(("pt", p, ui))
                if pi == NCHUNK - 1:
                    state.pop(("grp", g))

        def emit_drains(p, parts):
            """compact psO -> per-problem out tile (fp16), DMA when done."""
            if parts[0] == 0:
                state[("outp", p)] = outp.tile(
                    [128, POW], F16, name="outt", tag="outt"
                )
            outg = state[("outp", p)]
            pso = state[("psO", p)]
            for i in parts:
                s0, s1, d0 = DRAINS[i]
                nc.vector.tensor_copy(
                    out=outg[:, d0: d0 + (s1 - s0)], in_=pso[:, s0:s1]
                )
            if parts[-1] == len(DRAINS) - 1:
                nc.sync.dma_start(
                    out=outT.ap()[:, p * POW:(p + 1) * POW], in_=outg
                )
                state.pop(("outp", p))
                state.pop(("psO", p))

        # software-pipelined main loop. Per step p (engine queue order):
        #   PE : scores uA(p), uB(p) | PV(p-1) qb0-5 | scores uC(p) |
        #        PV(p-1) qb6-7 | scores uD(p)
        #   ACT: exp uA(p), uB(p), uC(p), uD(p)
        #   DVE: drains(p-1), masks(p)
        for p in range(nprob + 1):
            if p < nprob:
                if p == 0:
                    emit_loads(0, fine=True)
                if p % NCHUNK == 1 and (p // NCHUNK) + 1 < ngroups:
                    emit_loads(p // NCHUNK + 1)
                emit_scores_exp_unit(p, 0)
                emit_scores_exp_unit(p, 1)
            if p > 0:
                emit_pv(p - 1, [0, 1, 2, 3, 4, 5])
                emit_drains(p - 1, [0, 1])
            if p < nprob:
                emit_masks(p, [0, 1])
                emit_scores_exp_unit(p, 2)
            if p > 0:
                emit_pv(p - 1, [6, 7])
                emit_drains(p - 1, [2])
            if p < nprob:
                emit_scores_exp_unit(p, 3)
                emit_masks(p, [2, 3])


def _host_consts():
    freqs = np.exp(np.arange(HALF, dtype=np.float64) * (-math.log(ROPE_BASE) / HALF))
    pos = np.arange(L, dtype=np.float64)
    ang = pos[:, None] * freqs[None, :]  # (L, 64)
    cos = np.cos(ang)
    sin = np.sin(ang)
    r = np.arange(128)
    tri = (r[None, :] >= r[:, None]).astype(np.float16)  # keep q >= k
    return cos, sin, tri


def _rope_host(x, cos, sin):
    """x: (B, L, HPC, 128) fp32; cos/sin: (L, 64)."""
    x1, x2 = x[..., :HALF], x[..., HALF:]
    c = cos[None, :, None, :]
    s = sin[None, :, None, :]
    return np.concatenate([x1 * c - x2 * s, x2 * c + x1 * s], axis=-1)


def _pack_core(qc, kc, vc, cos, sin):
    """qc,kc,vc: (B, L, HPC, 128) fp32 -> device input maps."""
    qc = _rope_host(qc, cos, sin)
    kc = _rope_host(kc, cos, sin)

    def dmaj(x):
        # (B, L, h, D) -> (b, h, n, j, d) -> (d, b, h, n, j)
        a = x.transpose(0, 2, 1, 3).reshape(B, HPC, NCHUNK, CHUNK, DH)
        a = a.transpose(4, 0, 1, 2, 3).reshape(DH, NPROB * CHUNK)
        return np.ascontiguousarray(a).astype(np.float16)

    # v: partition = k-within-block, cols = (b,h,n, block, dv|1|pad3)
    a = vc.transpose(0, 2, 1, 3).reshape(B, HPC, NCHUNK, NB, 128, DV)
    ext = np.zeros(a.shape[:-1] + (VS,), a.dtype)
    ext[..., :DV] = a
    ext[..., DV] = 1.0
    ext = ext.transpose(4, 0, 1, 2, 3, 5).reshape(128, NPROB * PW)
    vp = np.ascontiguousarray(ext).astype(np.float16)
    return dict(qT_in=dmaj(qc), kT_in=dmaj(kc), vT_in=vp)


_NC_CACHE = {}
LAST_RESULT = None


def _get_module(nprob=NPROB):
    if nprob not in _NC_CACHE:
        _NC_CACHE[nprob] = build_module(nprob)
    return _NC_CACHE[nprob]


def kernel(q, k, v):
    q = np.asarray(q, dtype=np.float32)
    k = np.asarray(k, dtype=np.float32)
    v = np.asarray(v, dtype=np.float32)

    cos, sin, tri = _host_consts()
    consts = dict(tri_in=tri)

    in_maps = []
    for c in range(NCORES):
        hs = slice(HPC * c, HPC * (c + 1))
        m = _pack_core(q[:, :, hs], k[:, :, hs], v[:, :, hs], cos, sin)
        m.update(consts)
        in_maps.append(m)

    nc = _get_module(NPROB)
    trace = bool(int(os.environ.get("KERNEL_TRACE", "0")))
    res = bass_utils.run_bass_kernel_spmd(
        nc, in_maps, core_ids=list(range(NCORES)), trace=trace
    )
    global LAST_RESULT
    LAST_RESULT = res

    out = np.empty((B, L, H, DV), np.float32)
    for c in range(NCORES):
        ot = res.results[c]["outT_out"].astype(np.float32)  # (128q, 32*1040)
        # cols: (p, qb, d) with d in [0,130); partitions: q within q-block
        o = ot.reshape(128, NPROB, NB, OW)
        num = o[..., :DV]  # (q, p, qb, dv)
        den = o[..., DV]  # (q, p, qb)
        r = num / den[..., None]
        # (q, p, qb, dv) -> (p, qb, q, dv) -> (b, h, n, l_in_chunk, dv)
        r = r.transpose(1, 2, 0, 3).reshape(B, HPC, NCHUNK, CHUNK, DV)
        # -> (b, n, j, h, dv) -> (b, L, h, dv)
        r = r.transpose(0, 2, 3, 1, 4).reshape(B, L, HPC, DV)
        out[:, :, HPC * c:HPC * (c + 1)] = r
    return out
